# revision 18
# baseline (speedup 1.0000x reference)
"""Grouped-GEMM MoE experts (E=64, H=2048, F=1408, 16 tokens/expert, SwiGLU),
expert-parallel across 8 Trainium2 NeuronCores.

Per core: 8 experts, ~277 MB of fp32 weights streamed from HBM once.  All
weights are cast fp32->bf16 *inside the DMA* (SWDGE casts on the fly): HBM
traffic is unchanged (fp32 reads = the roofline), but TensorE runs native
bf16 single-pass matmuls instead of the 2x-cost fp32 LOW_HIGH path, putting
the kernel firmly in the DMA-bound regime.  PSUM accumulation stays fp32;
tolerance is 2e-2 and bf16 rounding contributes ~4e-3.

Since the contraction order over H (and F for down-proj) is free, chunks are
remapped so every DMA reads long CONTIGUOUS runs per partition:

  gate/up: chunk c, partition p  <->  row h = p*16 + c   => partition p reads
           rows p*16+c0 .. p*16+c0+CG-1 back-to-back (CG*5632 B contig).
  down:    chunk fc, partition q <->  row f = q*11 + fc  => partition q reads
           all 11 of its w2 rows in one DMA (90 KB contig).

The gate/up weight F-columns are sliced with stride 11 (f = q*11 + fc) so the
gate/up PSUM output lands directly in the f-layout the down-proj lhsT needs.
The x transpose uses stride-16 column slices to build xT in the matching
interleaved H-layout.

  gateT/upT [f,tok] = W1/W3 chunk.T @ xT chunk   (lhsT = weight, rhs = xT)
  interT = silu(gateT) * upT                      (bf16, already [f, tok])
  out[tok,h]  = interT chunk.T @ W2 chunk         (lhsT = interT, rhs = W2)
"""

import sys

if "/opt/trn_rl_repo" not in sys.path:
    sys.path.insert(0, "/opt/trn_rl_repo")

import numpy as np

E, H, F = 64, 2048, 1408
TOK = 16                  # tokens per expert (uniform routing)
NCORES = 8
E_LOC = E // NCORES       # 8 experts per core
T_LOC = E_LOC * TOK       # 128 tokens per core
P = 128
HC = H // P               # 16 contraction chunks for gate/up
FC = F // P               # 11 contraction chunks for down
CG = 8                    # H-chunks per gate/up weight DMA slot (5.77 MB HBM each)
NFREE = 512               # one PSUM bank of fp32
NT = H // NFREE           # 4 down-proj output tiles


def _gu_splits(e):
    """(start_chunk, n_chunks) per gate/up DMA.  The last expert tapers so
    the final tile's matmul burst fits under the trailing w2 DMA stream and
    the post-last-byte tail stays short."""
    if e < E_LOC - 1:
        return [(0, 8), (8, 8)]
    return [(0, 8), (8, 4), (12, 2), (14, 2)]


def _w2_splits(e):
    if e < E_LOC - 1:
        return [(0, 11)]
    return [(0, 6), (6, 3), (9, 1), (10, 1)]

_cache = {}


def _build_nc():
    import concourse.mybir as mybir
    from concourse import bacc
    from concourse.tile import TileContext
    from concourse.masks import make_identity

    f32 = mybir.dt.float32
    bf16 = mybir.dt.bfloat16
    AF = mybir.ActivationFunctionType

    # Bacc (not raw Bass): its finalize() pass splits multi-sem waits into
    # event-semaphore instructions — walrus allows only one wait per
    # hardware instruction.
    nc = bacc.Bacc()
    x_d = nc.declare_dram_parameter("x", [T_LOC, H], f32, isOutput=False)
    w1_d = nc.declare_dram_parameter("w1", [E_LOC, H, F], f32, isOutput=False)
    w3_d = nc.declare_dram_parameter("w3", [E_LOC, H, F], f32, isOutput=False)
    w2_d = nc.declare_dram_parameter("w2", [E_LOC, F, H], f32, isOutput=False)
    y_d = nc.declare_dram_parameter("y", [T_LOC, H], f32, isOutput=True)

    with TileContext(nc) as tc:
        with (
            tc.tile_pool(name="const", bufs=1) as constp,
            tc.tile_pool(name="xs", bufs=1) as xs,
            tc.tile_pool(name="wgu", bufs=2) as wgu,
            tc.tile_pool(name="w2p", bufs=2) as w2p,
            tc.tile_pool(name="acts", bufs=2) as acts,
            tc.tile_pool(name="obp", bufs=1) as obp,
            tc.tile_pool(name="ps_gu", bufs=2, space="PSUM") as ps_gu,
            tc.tile_pool(name="ps_dn", bufs=1, space="PSUM") as ps_dn,
        ):
            # Issue expert 0's first weight DMAs ahead of everything else on
            # the gpsimd queue so the HBM stream starts at t~1us.
            pre_w1 = wgu.tile([P, CG, F], bf16, tag="w1")
            pre_w3 = wgu.tile([P, CG, F], bf16, tag="w3")
            nc.gpsimd.dma_start(
                out=pre_w1[:],
                in_=w1_d[0].rearrange("(p c) f -> p c f", p=P)[:, 0:CG, :],
            )
            nc.gpsimd.dma_start(
                out=pre_w3[:],
                in_=w3_d[0].rearrange("(p c) f -> p c f", p=P)[:, 0:CG, :],
            )

            ident = constp.tile([P, P], bf16)
            make_identity(nc, ident[:])

            # x: fp32 via HWDGE (keeps the gpsimd weight queue free), cast on
            # DVE, then 16 strided transposes into the interleaved layout
            # xT[p, c*128 + t] = x[t, p*16 + c].
            x32 = xs.tile([P, H], f32)
            nc.sync.dma_start(out=x32[:], in_=x_d[:, :])
            x_sb = xs.tile([P, H], bf16)
            nc.vector.tensor_copy(out=x_sb[:], in_=x32[:])
            xr = x_sb[:].rearrange("t (hp s) -> t s hp", s=HC)
            xT = xs.tile([P, HC * T_LOC], bf16)
            for g in range(HC // 4):
                pt = ps_dn.tile([P, H], f32, tag="dn")
                for j in range(4):
                    c = 4 * g + j
                    nc.tensor.matmul(
                        pt[:, j * P : (j + 1) * P],
                        xr[:, c, :],
                        ident[:],
                        start=True,
                        stop=True,
                    )
                nc.vector.tensor_copy(
                    out=xT[:, g * 512 : (g + 1) * 512], in_=pt[:, :512]
                )

            def make_dn_emitter(e, it, w2_parts):
                """Deferred down-proj for expert e: emitted on the Tensor
                queue only after expert e+1's first gate/up block, so the
                in-order Tensor queue keeps consuming (and freeing) gate/up
                weight tiles while silu/mul and the w2 loads settle.  Without
                this, 44 down matmuls head-of-line-block the next expert's
                gate/up consumption and the weight DMA queue stalls on slot
                semaphores every expert."""

                def emit():
                    dn = ps_dn.tile([P, H], f32, tag="dn")
                    for fc0, cw, w2t in w2_parts:
                        for ci in range(cw):
                            fc = fc0 + ci
                            for nt in range(NT):
                                nc.tensor.matmul(
                                    dn[:TOK, nt * NFREE : (nt + 1) * NFREE],
                                    it[:, fc * TOK : (fc + 1) * TOK],
                                    w2t[:, ci, nt * NFREE : (nt + 1) * NFREE],
                                    start=(fc == 0),
                                    stop=(fc == FC - 1),
                                )
                    ob = obp.tile([TOK, H], f32, tag="ob")
                    nc.vector.tensor_copy(out=ob[:], in_=dn[:TOK, :])
                    nc.sync.dma_start(
                        out=y_d[e * TOK : (e + 1) * TOK, :], in_=ob[:]
                    )

                return emit

            dn_emit = None
            for e in range(E_LOC):
                # gate/up accumulation: all FC output chunks share one PSUM
                # bank per tensor.  Only the very first matmul into the bank
                # carries start=True (clears the whole bank's has_written
                # bits); every other chunk's first write then overwrites via
                # the per-element has_written mechanism.
                gt = ps_gu.tile([P, FC * TOK], f32, tag="gt")
                ut = ps_gu.tile([P, FC * TOK], f32, tag="ut")
                w1_ap = w1_d[e].rearrange("(p c) f -> p c f", p=P)
                w3_ap = w3_d[e].rearrange("(p c) f -> p c f", p=P)
                for si, (c0, nch) in enumerate(_gu_splits(e)):
                    if si == 1 and dn_emit is not None:
                        dn_emit()
                        dn_emit = None
                    if e == 0 and si == 0:
                        w1t, w3t = pre_w1, pre_w3
                    else:
                        w1t = wgu.tile([P, CG, F], bf16, tag="w1")
                        w3t = wgu.tile([P, CG, F], bf16, tag="w3")
                        cs = slice(c0, c0 + nch)
                        nc.gpsimd.dma_start(out=w1t[:, :nch, :], in_=w1_ap[:, cs, :])
                        nc.gpsimd.dma_start(out=w3t[:, :nch, :], in_=w3_ap[:, cs, :])
                    w1r = w1t[:].rearrange("p c (q s) -> p c s q", s=FC)
                    w3r = w3t[:].rearrange("p c (q s) -> p c s q", s=FC)
                    for ci in range(nch):
                        c = c0 + ci
                        rhs = xT[:, c * P + e * TOK : c * P + (e + 1) * TOK]
                        first = c == 0
                        last = c == HC - 1
                        for fc in range(FC):
                            nc.tensor.matmul(
                                gt[:, fc * TOK : (fc + 1) * TOK],
                                w1r[:, ci, fc, :],
                                rhs,
                                start=(first and fc == 0),
                                stop=(last and fc == FC - 1),
                                skip_group_check=True,
                            )
                        for fc in range(FC):
                            nc.tensor.matmul(
                                ut[:, fc * TOK : (fc + 1) * TOK],
                                w3r[:, ci, fc, :],
                                rhs,
                                start=(first and fc == 0),
                                stop=(last and fc == FC - 1),
                                skip_group_check=True,
                            )

                gs = acts.tile([P, FC * TOK], f32, tag="gs")
                it = acts.tile([P, FC * TOK], bf16, tag="it")
                nc.scalar.activation(gs[:], gt[:], AF.Silu)
                nc.vector.tensor_mul(it[:], gs[:], ut[:])

                w2_ap = w2_d[e].rearrange("(p c) h -> p c h", p=P)
                w2_parts = []
                for fc0, cw in _w2_splits(e):
                    w2t = w2p.tile([P, 11, H], bf16, tag="w2")
                    nc.gpsimd.dma_start(
                        out=w2t[:, :cw, :], in_=w2_ap[:, fc0 : fc0 + cw, :]
                    )
                    w2_parts.append((fc0, cw, w2t))
                dn_emit = make_dn_emitter(e, it, w2_parts)

            dn_emit()

    if not nc.is_finalized():
        nc.finalize()
    return nc


def _get_nc():
    if "nc" not in _cache:
        _cache["nc"] = _build_nc()
    return _cache["nc"]


def _make_in_maps(inputs):
    x = np.ascontiguousarray(np.asarray(inputs["permuted_local_hidden_states"], dtype=np.float32))
    w1 = np.ascontiguousarray(np.asarray(inputs["gate_proj"], dtype=np.float32))
    w3 = np.ascontiguousarray(np.asarray(inputs["up_proj"], dtype=np.float32))
    w2 = np.ascontiguousarray(np.asarray(inputs["down_proj"], dtype=np.float32))
    in_maps = []
    for m in range(NCORES):
        in_maps.append(
            {
                "x": x[m * T_LOC : (m + 1) * T_LOC],
                "w1": w1[m * E_LOC : (m + 1) * E_LOC],
                "w3": w3[m * E_LOC : (m + 1) * E_LOC],
                "w2": w2[m * E_LOC : (m + 1) * E_LOC],
            }
        )
    return in_maps


def run(inputs, trace=False, **kwargs):
    """Run the SPMD kernel; returns (y_full, BassKernelResults)."""
    from concourse.bass_utils import run_bass_kernel_spmd

    nc = _get_nc()
    res = run_bass_kernel_spmd(
        nc, _make_in_maps(inputs), list(range(NCORES)), trace=trace, **kwargs
    )
    y = np.concatenate([res.results[m]["y"] for m in range(NCORES)], axis=0)
    return y.astype(np.float32, copy=False), res


def kernel(**inputs):
    y, _ = run(inputs, trace=False)
    return y


# revision 20
# speedup vs baseline: 1.0072x; 1.0072x over previous
"""Grouped-GEMM MoE experts (E=64, H=2048, F=1408, 16 tokens/expert, SwiGLU),
expert-parallel across 8 Trainium2 NeuronCores.

Per core: 8 experts, ~277 MB of fp32 weights streamed from HBM once.  All
weights are cast fp32->bf16 *inside the DMA* (SWDGE casts on the fly): HBM
traffic is unchanged (fp32 reads = the roofline), but TensorE runs native
bf16 single-pass matmuls instead of the 2x-cost fp32 LOW_HIGH path, putting
the kernel firmly in the DMA-bound regime.  PSUM accumulation stays fp32;
tolerance is 2e-2 and bf16 rounding contributes ~4e-3.

Since the contraction order over H (and F for down-proj) is free, chunks are
remapped so every DMA reads long CONTIGUOUS runs per partition:

  gate/up: chunk c, partition p  <->  row h = p*16 + c   => partition p reads
           rows p*16+c0 .. p*16+c0+CG-1 back-to-back (CG*5632 B contig).
  down:    chunk fc, partition q <->  row f = q*11 + fc  => partition q reads
           all 11 of its w2 rows in one DMA (90 KB contig).

The gate/up weight F-columns are sliced with stride 11 (f = q*11 + fc) so the
gate/up PSUM output lands directly in the f-layout the down-proj lhsT needs.
The x transpose uses stride-16 column slices to build xT in the matching
interleaved H-layout.

  gateT/upT [f,tok] = W1/W3 chunk.T @ xT chunk   (lhsT = weight, rhs = xT)
  interT = silu(gateT) * upT                      (bf16, already [f, tok])
  out[tok,h]  = interT chunk.T @ W2 chunk         (lhsT = interT, rhs = W2)
"""

import sys

if "/opt/trn_rl_repo" not in sys.path:
    sys.path.insert(0, "/opt/trn_rl_repo")

import numpy as np

E, H, F = 64, 2048, 1408
TOK = 16                  # tokens per expert (uniform routing)
NCORES = 8
E_LOC = E // NCORES       # 8 experts per core
T_LOC = E_LOC * TOK       # 128 tokens per core
P = 128
HC = H // P               # 16 contraction chunks for gate/up
FC = F // P               # 11 contraction chunks for down
CG = 8                    # H-chunks per gate/up weight DMA slot (5.77 MB HBM each)
NFREE = 512               # one PSUM bank of fp32
NT = H // NFREE           # 4 down-proj output tiles


def _gu_splits(e):
    """(start_chunk, n_chunks) per gate/up DMA.  The last expert tapers so
    the final tile's matmul burst fits under the trailing w2 DMA stream and
    the post-last-byte tail stays short."""
    if e < E_LOC - 1:
        return [(0, 8), (8, 8)]
    return [(0, 8), (8, 4), (12, 4)]


def _w2_splits(e):
    if e < E_LOC - 1:
        return [(0, 11)]
    return [(0, 6), (6, 4), (10, 1)]

_cache = {}


def _build_nc():
    import concourse.mybir as mybir
    from concourse import bacc
    from concourse.tile import TileContext
    from concourse.masks import make_identity

    f32 = mybir.dt.float32
    bf16 = mybir.dt.bfloat16
    AF = mybir.ActivationFunctionType

    # Bacc (not raw Bass): its finalize() pass splits multi-sem waits into
    # event-semaphore instructions — walrus allows only one wait per
    # hardware instruction.
    nc = bacc.Bacc()
    x_d = nc.declare_dram_parameter("x", [T_LOC, H], f32, isOutput=False)
    w1_d = nc.declare_dram_parameter("w1", [E_LOC, H, F], f32, isOutput=False)
    w3_d = nc.declare_dram_parameter("w3", [E_LOC, H, F], f32, isOutput=False)
    w2_d = nc.declare_dram_parameter("w2", [E_LOC, F, H], f32, isOutput=False)
    y_d = nc.declare_dram_parameter("y", [T_LOC, H], f32, isOutput=True)

    with TileContext(nc) as tc:
        with (
            tc.tile_pool(name="const", bufs=1) as constp,
            tc.tile_pool(name="xs", bufs=1) as xs,
            tc.tile_pool(name="wgu", bufs=2) as wgu,
            tc.tile_pool(name="w2p", bufs=2) as w2p,
            tc.tile_pool(name="acts", bufs=2) as acts,
            tc.tile_pool(name="obp", bufs=1) as obp,
            tc.tile_pool(name="ps_gu", bufs=2, space="PSUM") as ps_gu,
            tc.tile_pool(name="ps_dn", bufs=1, space="PSUM") as ps_dn,
        ):
            # Issue expert 0's first weight DMAs ahead of everything else on
            # the gpsimd queue so the HBM stream starts at t~1us.
            pre_w1 = wgu.tile([P, CG, F], bf16, tag="w1")
            pre_w3 = wgu.tile([P, CG, F], bf16, tag="w3")
            nc.gpsimd.dma_start(
                out=pre_w1[:],
                in_=w1_d[0].rearrange("(p c) f -> p c f", p=P)[:, 0:CG, :],
            )
            nc.gpsimd.dma_start(
                out=pre_w3[:],
                in_=w3_d[0].rearrange("(p c) f -> p c f", p=P)[:, 0:CG, :],
            )

            ident = constp.tile([P, P], bf16)
            make_identity(nc, ident[:])

            # x: fp32 via HWDGE (keeps the gpsimd weight queue free), cast on
            # DVE, then 16 strided transposes into the interleaved layout
            # xT[p, c*128 + t] = x[t, p*16 + c].
            x32 = xs.tile([P, H], f32)
            nc.sync.dma_start(out=x32[:], in_=x_d[:, :])
            x_sb = xs.tile([P, H], bf16)
            nc.vector.tensor_copy(out=x_sb[:], in_=x32[:])
            xr = x_sb[:].rearrange("t (hp s) -> t s hp", s=HC)
            xT = xs.tile([P, HC * T_LOC], bf16)
            for g in range(HC // 4):
                pt = ps_dn.tile([P, H], f32, tag="dn")
                for j in range(4):
                    c = 4 * g + j
                    nc.tensor.matmul(
                        pt[:, j * P : (j + 1) * P],
                        xr[:, c, :],
                        ident[:],
                        start=True,
                        stop=True,
                    )
                nc.vector.tensor_copy(
                    out=xT[:, g * 512 : (g + 1) * 512], in_=pt[:, :512]
                )

            def make_dn_emitter(e, it, w2_parts):
                """Deferred down-proj for expert e: emitted on the Tensor
                queue only after expert e+1's first gate/up block, so the
                in-order Tensor queue keeps consuming (and freeing) gate/up
                weight tiles while silu/mul and the w2 loads settle.  Without
                this, 44 down matmuls head-of-line-block the next expert's
                gate/up consumption and the weight DMA queue stalls on slot
                semaphores every expert."""

                def emit():
                    dn = ps_dn.tile([P, H], f32, tag="dn")
                    for fc0, cw, w2t in w2_parts:
                        for ci in range(cw):
                            fc = fc0 + ci
                            for nt in range(NT):
                                nc.tensor.matmul(
                                    dn[:TOK, nt * NFREE : (nt + 1) * NFREE],
                                    it[:, fc * TOK : (fc + 1) * TOK],
                                    w2t[:, ci, nt * NFREE : (nt + 1) * NFREE],
                                    start=(fc == 0),
                                    stop=(fc == FC - 1),
                                )
                    ob = obp.tile([TOK, H], f32, tag="ob")
                    nc.vector.tensor_copy(out=ob[:], in_=dn[:TOK, :])
                    nc.sync.dma_start(
                        out=y_d[e * TOK : (e + 1) * TOK, :], in_=ob[:]
                    )

                return emit

            dn_emit = None
            for e in range(E_LOC):
                # gate/up accumulation: all FC output chunks share one PSUM
                # bank per tensor.  Only the very first matmul into the bank
                # carries start=True (clears the whole bank's has_written
                # bits); every other chunk's first write then overwrites via
                # the per-element has_written mechanism.
                gt = ps_gu.tile([P, FC * TOK], f32, tag="gt")
                ut = ps_gu.tile([P, FC * TOK], f32, tag="ut")
                w1_ap = w1_d[e].rearrange("(p c) f -> p c f", p=P)
                w3_ap = w3_d[e].rearrange("(p c) f -> p c f", p=P)
                for si, (c0, nch) in enumerate(_gu_splits(e)):
                    if si == 1 and dn_emit is not None:
                        dn_emit()
                        dn_emit = None
                    if e == 0 and si == 0:
                        w1t, w3t = pre_w1, pre_w3
                    else:
                        w1t = wgu.tile([P, CG, F], bf16, tag="w1")
                        w3t = wgu.tile([P, CG, F], bf16, tag="w3")
                        cs = slice(c0, c0 + nch)
                        nc.gpsimd.dma_start(out=w1t[:, :nch, :], in_=w1_ap[:, cs, :])
                        nc.gpsimd.dma_start(out=w3t[:, :nch, :], in_=w3_ap[:, cs, :])
                    w1r = w1t[:].rearrange("p c (q s) -> p c s q", s=FC)
                    w3r = w3t[:].rearrange("p c (q s) -> p c s q", s=FC)
                    for ci in range(nch):
                        c = c0 + ci
                        rhs = xT[:, c * P + e * TOK : c * P + (e + 1) * TOK]
                        first = c == 0
                        last = c == HC - 1
                        for fc in range(FC):
                            nc.tensor.matmul(
                                gt[:, fc * TOK : (fc + 1) * TOK],
                                w1r[:, ci, fc, :],
                                rhs,
                                start=(first and fc == 0),
                                stop=(last and fc == FC - 1),
                                skip_group_check=True,
                            )
                        for fc in range(FC):
                            nc.tensor.matmul(
                                ut[:, fc * TOK : (fc + 1) * TOK],
                                w3r[:, ci, fc, :],
                                rhs,
                                start=(first and fc == 0),
                                stop=(last and fc == FC - 1),
                                skip_group_check=True,
                            )

                gs = acts.tile([P, FC * TOK], f32, tag="gs")
                it = acts.tile([P, FC * TOK], bf16, tag="it")
                nc.scalar.activation(gs[:], gt[:], AF.Silu)
                nc.vector.tensor_mul(it[:], gs[:], ut[:])

                w2_ap = w2_d[e].rearrange("(p c) h -> p c h", p=P)
                w2_parts = []
                for fc0, cw in _w2_splits(e):
                    w2t = w2p.tile([P, 11, H], bf16, tag="w2")
                    nc.gpsimd.dma_start(
                        out=w2t[:, :cw, :], in_=w2_ap[:, fc0 : fc0 + cw, :]
                    )
                    w2_parts.append((fc0, cw, w2t))
                dn_emit = make_dn_emitter(e, it, w2_parts)

            dn_emit()

    if not nc.is_finalized():
        nc.finalize()
    return nc


def _get_nc():
    if "nc" not in _cache:
        _cache["nc"] = _build_nc()
    return _cache["nc"]


def _make_in_maps(inputs):
    x = np.ascontiguousarray(np.asarray(inputs["permuted_local_hidden_states"], dtype=np.float32))
    w1 = np.ascontiguousarray(np.asarray(inputs["gate_proj"], dtype=np.float32))
    w3 = np.ascontiguousarray(np.asarray(inputs["up_proj"], dtype=np.float32))
    w2 = np.ascontiguousarray(np.asarray(inputs["down_proj"], dtype=np.float32))
    in_maps = []
    for m in range(NCORES):
        in_maps.append(
            {
                "x": x[m * T_LOC : (m + 1) * T_LOC],
                "w1": w1[m * E_LOC : (m + 1) * E_LOC],
                "w3": w3[m * E_LOC : (m + 1) * E_LOC],
                "w2": w2[m * E_LOC : (m + 1) * E_LOC],
            }
        )
    return in_maps


def run(inputs, trace=False, **kwargs):
    """Run the SPMD kernel; returns (y_full, BassKernelResults)."""
    from concourse.bass_utils import run_bass_kernel_spmd

    nc = _get_nc()
    res = run_bass_kernel_spmd(
        nc, _make_in_maps(inputs), list(range(NCORES)), trace=trace, **kwargs
    )
    y = np.concatenate([res.results[m]["y"] for m in range(NCORES)], axis=0)
    return y.astype(np.float32, copy=False), res


def kernel(**inputs):
    y, _ = run(inputs, trace=False)
    return y
